# revision 8
# baseline (speedup 1.0000x reference)
"""Arcee decoder layer on 8 TRN2 NeuronCores — tensor-parallel Bass kernel.

Sharding (8-way TP, transposed activation layout [hidden, seq] on device):
  - core c owns: q heads 4c..4c+3 + kv head c (GQA group), residual-stream
    rows 512c..512c+512, intermediate cols 2048c..2048c+2048.
  - RMSNorms run on the hid-sharded transposed stream: per-core partial
    sum-of-squares [1,S] + tiny AllReduce, then the normalized shard is
    AllGathered (bf16) to form the full transposed activation for TP matmuls.
  - o_proj / down_proj emit transposed partials [4096,S]; seq-chunked
    ReduceScatter gives each core its hid-slice of the sum = its slice of the
    transposed outputs. Host reassembles by concat + transpose.
  - dtypes: all matmuls bf16 (f32 PSUM accumulation); residual stream,
    norms and softmax statistics in f32.
"""
import sys

sys.path.insert(0, "/opt/trn_rl_repo")

import math
import numpy as np

import concourse.bass as bass
import concourse.mybir as mybir
import concourse.tile as tile
from concourse import bacc
from concourse.masks import make_identity

F32 = mybir.dt.float32
F32R = mybir.dt.float32r
BF16 = mybir.dt.bfloat16
I32 = mybir.dt.int32
AF = mybir.ActivationFunctionType
ALU = mybir.AluOpType

N_CORES = 8
S = 2048
HID = 4096
N_HEADS = 32
N_KV = 8
DHEAD = 128
INTER = 16384
EPS = 1e-5
THETA = 10000.0

HQ = N_HEADS // N_CORES          # 4 q heads per core
HID_SH = HID // N_CORES          # 512 residual rows per core
INT_SH = INTER // N_CORES        # 2048 intermediate per core
NJ = HQ + 2                      # qkv col tiles per core (4q + k + v)
QKV_COLS = NJ * DHEAD            # 768
P = 128
SC = 512                         # seq chunk (matmul moving dim)
NSC = S // SC                    # 4
NT_HID = HID // P                # 32
NT_HSH = HID_SH // P             # 4
NT_INT = INT_SH // P             # 16
TWO_PI = 2.0 * math.pi


def build_graph():
    nc = bacc.Bacc(None, target_bir_lowering=False, debug=False)

    hT = nc.declare_dram_parameter("hT", [HID_SH, S], F32, isOutput=False)
    rT = nc.declare_dram_parameter("rT", [HID_SH, S], F32, isOutput=False)
    pos_in = nc.declare_dram_parameter("positions", [1, S], I32, isOutput=False)
    wqkv = nc.declare_dram_parameter("wqkv", [HID, QKV_COLS], F32, isOutput=False)
    wo = nc.declare_dram_parameter("wo", [HQ * DHEAD, HID], F32, isOutput=False)
    wup = nc.declare_dram_parameter("wup", [HID, INT_SH], F32, isOutput=False)
    wdn = nc.declare_dram_parameter("wdn", [INT_SH, HID], F32, isOutput=False)
    ln1 = nc.declare_dram_parameter("ln1", [P, NT_HSH], F32, isOutput=False)
    ln2 = nc.declare_dram_parameter("ln2", [P, NT_HSH], F32, isOutput=False)
    out_res2 = nc.declare_dram_parameter("res2T", [HID_SH, S], F32, isOutput=True)
    out_mlp = nc.declare_dram_parameter("mlpT", [HID_SH, S], F32, isOutput=True)

    RG = [list(range(N_CORES))]
    inv_sqrt_d = 1.0 / math.sqrt(DHEAD)

    with tile.TileContext(nc) as tc:
        import contextlib
        with contextlib.ExitStack() as ctx:
            const = ctx.enter_context(tc.tile_pool(name="const", bufs=1))
            rowsb = ctx.enter_context(tc.tile_pool(name="rowsb", bufs=1))
            acc = ctx.enter_context(tc.tile_pool(name="acc", bufs=5, space="PSUM"))
            rowps = ctx.enter_context(tc.tile_pool(name="rowps", bufs=2, space="PSUM"))
            tpps = ctx.enter_context(tc.tile_pool(name="tpps", bufs=1, space="PSUM"))
            dram = ctx.enter_context(tc.tile_pool(name="dram", bufs=1, space="DRAM"))

            # ============ constants ============
            ident = const.tile([P, P], BF16)
            make_identity(nc, ident[:])
            ones_bf = const.tile([P, 1], BF16)
            nc.vector.memset(ones_bf[:], 1.0)
            ln1_sb = const.tile([P, NT_HSH], F32)
            ln2_sb = const.tile([P, NT_HSH], F32)
            nc.sync.dma_start(ln1_sb[:], ln1[:])
            nc.sync.dma_start(ln2_sb[:], ln2[:])
            cos2 = const.tile([P, S], BF16)
            sin_neg = const.tile([P, S], BF16)

            # ============ rope tables (scoped scratch) ============
            with tc.tile_pool(name="tbl", bufs=1) as tbl:
                iot = tbl.tile([64, 1], I32)
                nc.gpsimd.iota(iot[:], pattern=[[1, 1]], base=0, channel_multiplier=1)
                iotf = tbl.tile([64, 1], F32)
                nc.vector.tensor_copy(iotf[:], iot[:])
                invf = tbl.tile([64, 1], F32)
                nc.scalar.activation(invf[:], iotf[:], AF.Exp,
                                     scale=-math.log(THETA) / 64.0)
                invf2pi = tbl.tile([64, 1], F32)
                nc.scalar.activation(invf2pi[:], invf[:], AF.Copy,
                                     scale=1.0 / TWO_PI)
                posi = tbl.tile([1, S], I32)
                nc.sync.dma_start(posi[:], pos_in[:])
                posf = tbl.tile([1, S], F32)
                nc.vector.tensor_copy(posf[:], posi[:])
                posb = tbl.tile([64, S], F32)
                nc.gpsimd.partition_broadcast(posb[:], posf[:])

                def range_reduce_sin(dst_bf, t_ap, negate=False):
                    # dst = sin(2*pi*t) via two-stage round-and-subtract
                    n1 = tbl.tile([64, S], I32, tag="rri", bufs=2)
                    nc.vector.tensor_copy(n1[:], t_ap)
                    n1f = tbl.tile([64, S], F32, tag="rrf", bufs=2)
                    nc.vector.tensor_copy(n1f[:], n1[:])
                    f1 = tbl.tile([64, S], F32, tag="rrg", bufs=2)
                    nc.vector.tensor_tensor(f1[:], t_ap, n1f[:], ALU.subtract)
                    n2 = tbl.tile([64, S], I32, tag="rri", bufs=2)
                    nc.vector.tensor_copy(n2[:], f1[:])
                    n2f = tbl.tile([64, S], F32, tag="rrf", bufs=2)
                    nc.vector.tensor_copy(n2f[:], n2[:])
                    f2 = tbl.tile([64, S], F32, tag="rrg", bufs=2)
                    nc.vector.tensor_tensor(f2[:], f1[:], n2f[:], ALU.subtract)
                    nc.scalar.activation(dst_bf, f2[:], AF.Sin,
                                         scale=-TWO_PI if negate else TWO_PI)

                tfrac = tbl.tile([64, S], F32)
                nc.scalar.activation(tfrac[:], posb[:], AF.Copy, scale=invf2pi[:])
                sinb = tbl.tile([64, S], BF16)       # +sin
                sinnb = tbl.tile([64, S], BF16)      # -sin
                range_reduce_sin(sinb[:], tfrac[:])
                range_reduce_sin(sinnb[:], tfrac[:], negate=True)
                tfrac2 = tbl.tile([64, S], F32)
                nc.scalar.activation(tfrac2[:], tfrac[:], AF.Copy, bias=0.25)
                cosb = tbl.tile([64, S], BF16)
                range_reduce_sin(cosb[:], tfrac2[:])
                nc.sync.dma_start(cos2[:64, :], cosb[:])
                nc.sync.dma_start(cos2[64:, :], cosb[:])
                nc.sync.dma_start(sin_neg[:64, :], sinnb[:])
                nc.sync.dma_start(sin_neg[64:, :], sinb[:])

                # ============ weight bf16 caches in DRAM ("SBUF image":
                # row p holds partition p's contiguous per-tile data) ======
                wqkv_c = dram.tile([P, NT_HID * QKV_COLS], BF16, name="wqkv_c")
                wo_c = dram.tile([P, HQ * HID], BF16, name="wo_c")
                wup_c = dram.tile([P, NT_HID * INT_SH], BF16, name="wup_c")

                def build_cache(src, n_row_tiles, n_cols, dst):
                    for k in range(n_row_tiles):
                        wf = tbl.tile([P, n_cols], F32, tag="cbf", bufs=2)
                        nc.sync.dma_start(wf[:], src[k * P:(k + 1) * P, :])
                        wb = tbl.tile([P, n_cols], BF16, tag="cbb", bufs=2)
                        nc.gpsimd.tensor_copy(wb[:], wf[:])
                        nc.sync.dma_start(dst[:, k * n_cols:(k + 1) * n_cols], wb[:])

                build_cache(wqkv, NT_HID, QKV_COLS, wqkv_c)
                build_cache(wo, HQ, HID, wo_c)
                build_cache(wup, NT_HID, INT_SH, wup_c)

            wqkv_v = wqkv_c[:].rearrange("p (k c) -> p k c", k=NT_HID)
            wo_v = wo_c[:].rearrange("p (a c) -> p a c", a=HQ)
            wup_v = wup_c[:].rearrange("p (k c) -> p k c", k=NT_HID)

            # xT staging in DRAM (residual stream f32, reread per chunk)
            xT_d = dram.tile([HID_SH, S], F32, name="xT_d")

            ar1_in = dram.tile([1, S], F32, name="ar1_in")
            ar1_out = dram.tile([1, S], F32, name="ar1_out", addr_space="Shared")
            ag1_in = dram.tile([HID_SH, S], BF16, name="ag1_in")
            ag1_out = dram.tile([HID, S], BF16, name="ag1_out", addr_space="Shared")
            rs1_in = [dram.tile([HID, SC], F32, name=f"rs1_in{sc}") for sc in range(NSC)]
            rs1_out = [dram.tile([HID_SH, SC], F32, name=f"rs1_out{sc}") for sc in range(NSC)]
            ar2_in = [dram.tile([1, SC], F32, name=f"ar2_in{sc}") for sc in range(NSC)]
            ar2_out = [dram.tile([1, SC], F32, name=f"ar2_out{sc}", addr_space="Shared")
                       for sc in range(NSC)]
            ag2_in = [dram.tile([HID_SH, SC], BF16, name=f"ag2_in{sc}") for sc in range(NSC)]
            ag2_out = [dram.tile([HID, SC], BF16, name=f"ag2_out{sc}", addr_space="Shared")
                       for sc in range(NSC)]
            rs2_in = [dram.tile([HID, SC], F32, name=f"rs2_in{sc}") for sc in range(NSC)]
            rs2_out = [dram.tile([HID_SH, SC], F32, name=f"rs2_out{sc}") for sc in range(NSC)]

            # ================== attention era ==================
            with tc.tile_pool(name="apersist", bufs=1) as apersist, \
                 tc.tile_pool(name="awork", bufs=1) as awork, \
                 tc.tile_pool(name="wstr", bufs=1) as wstr:

                _cnt = [0]

                def t2k(tag="t2k", bufs=12):
                    _cnt[0] += 1
                    return awork.tile([P, SC], F32, tag=tag, bufs=bufs,
                                      name=f"t_{_cnt[0]}")

                def t1k(tag="t1k", bufs=8):
                    _cnt[0] += 1
                    return awork.tile([P, SC], BF16, tag=tag, bufs=bufs,
                                      name=f"t_{_cnt[0]}")

                def ssq_accum(ps, src_ap, first, last):
                    sq = t1k(tag="sq", bufs=6)
                    nc.scalar.activation(sq[:], src_ap, AF.Square)
                    nc.tensor.matmul(ps[:], ones_bf[:], sq[:],
                                     start=first, stop=last)

                # ---- phase 1: x = h + r (chunked), ssq1 -> AR -> scale1
                ssq1 = rowsb.tile([1, S], F32)
                for sc in range(NSC):
                    cs = slice(sc * SC, (sc + 1) * SC)
                    ps = rowps.tile([1, SC], F32, tag="row")
                    for i in range(NT_HSH):
                        a = t2k()
                        b = t2k()
                        nc.sync.dma_start(a[:], hT[i * P:(i + 1) * P, cs])
                        nc.sync.dma_start(b[:], rT[i * P:(i + 1) * P, cs])
                        xt = t2k()
                        nc.vector.tensor_tensor(xt[:], a[:], b[:], ALU.add)
                        nc.sync.dma_start(xT_d[i * P:(i + 1) * P, cs], xt[:])
                        ssq_accum(ps, xt[:], i == 0, i == NT_HSH - 1)
                    nc.vector.tensor_copy(ssq1[:, cs], ps[:])
                nc.sync.dma_start(ar1_in[:], ssq1[:])
                nc.gpsimd.collective_compute("AllReduce", ALU.add, replica_groups=RG,
                                             ins=[ar1_in[:].opt()], outs=[ar1_out[:].opt()])
                scale1 = rowsb.tile([1, S], F32)
                nc.sync.dma_start(scale1[:], ar1_out[:])
                nc.scalar.activation(scale1[:], scale1[:], AF.Copy,
                                     scale=1.0 / HID, bias=EPS)
                nc.vector.reciprocal(scale1[:], scale1[:])
                nc.scalar.activation(scale1[:], scale1[:], AF.Sqrt)
                scale1b = apersist.tile([P, S], F32)
                nc.gpsimd.partition_broadcast(scale1b[:], scale1[:])
                for sc in range(NSC):
                    cs = slice(sc * SC, (sc + 1) * SC)
                    for i in range(NT_HSH):
                        xr = t2k()
                        nc.sync.dma_start(xr[:], xT_d[i * P:(i + 1) * P, cs])
                        h1 = t2k()
                        nc.vector.tensor_tensor(h1[:], xr[:], scale1b[:, cs], ALU.mult)
                        h1b = t1k()
                        nc.scalar.activation(h1b[:], h1[:], AF.Copy,
                                             scale=ln1_sb[:, i:i + 1])
                        nc.sync.dma_start(ag1_in[i * P:(i + 1) * P, cs], h1b[:])
                nc.gpsimd.collective_compute("AllGather", ALU.bypass, replica_groups=RG,
                                             ins=[ag1_in[:].opt()], outs=[ag1_out[:].opt()])

                qkvT = [apersist.tile([P, S], BF16, name=f"qkvT{j}") for j in range(NJ)]
                attnT = [apersist.tile([P, S], BF16, name=f"attnT{a}") for a in range(HQ)]

                for sc in range(NSC):
                    cs = slice(sc * SC, (sc + 1) * SC)
                    # ---- qkv chunk
                    hg = []
                    for k in range(NT_HID):
                        g = awork.tile([P, SC], BF16, tag=f"hg{k}", bufs=1)
                        nc.sync.dma_start(g[:], ag1_out[k * P:(k + 1) * P, cs])
                        hg.append(g)
                    for j in range(NJ):
                        wj = wstr.tile([P, NT_HID, P], BF16, tag="wqkvs", bufs=2)
                        nc.sync.dma_start(wj[:], wqkv_v[:, :, j * P:(j + 1) * P])
                        ps = acc.tile([P, SC], F32, tag="acc")
                        for k in range(NT_HID):
                            nc.tensor.matmul(ps[:], wj[:, k, :], hg[k][:],
                                             start=(k == 0), stop=(k == NT_HID - 1))
                        nc.vector.tensor_copy(qkvT[j][:, cs], ps[:])

                    # ---- rope on q tiles and k tile (bf16, chunk cols)
                    for j in range(HQ + 1):
                        t = qkvT[j]
                        swp = t1k()
                        nc.sync.dma_start(swp[:64, :], t[64:, cs])
                        nc.sync.dma_start(swp[64:, :], t[:64, cs])
                        m1 = t1k()
                        nc.vector.tensor_tensor(m1[:], t[:, cs], cos2[:, cs], ALU.mult)
                        m2 = t1k()
                        nc.vector.tensor_tensor(m2[:], swp[:], sin_neg[:, cs], ALU.mult)
                        nc.vector.tensor_tensor(t[:, cs], m1[:], m2[:], ALU.add)

                    # ---- v transpose in place (block-transposed v)
                    for t in range(sc * (SC // P), (sc + 1) * (SC // P)):
                        pst = tpps.tile([P, P], BF16, tag="tp")
                        nc.tensor.transpose(pst[:], qkvT[NJ - 1][:, t * P:(t + 1) * P],
                                            ident[:])
                        nc.vector.tensor_copy(qkvT[NJ - 1][:, t * P:(t + 1) * P], pst[:])

                    # ---- attention (4 heads x this chunk)
                    nsk = (sc + 1) * (SC // P)
                    for h in range(HQ):
                        pv = acc.tile([P, SC], F32, tag="acc", name=f"pv{h}_{sc}")
                        rs = rowps.tile([1, SC], F32, tag="row", name=f"rs{h}_{sc}")
                        for skt in range(nsk):
                            sps = acc.tile([P, SC], F32, tag="acc",
                                           name=f"s{h}_{sc}_{skt}")
                            nc.tensor.matmul(sps[:],
                                             qkvT[HQ][:, skt * P:(skt + 1) * P],
                                             qkvT[h][:, cs], start=True, stop=True)
                            ex = t1k(tag="ex", bufs=6)
                            nc.scalar.activation(ex[:], sps[:], AF.Exp,
                                                 scale=inv_sqrt_d)
                            if skt >= 4 * sc:
                                nc.gpsimd.affine_select(
                                    ex[:], ex[:], pattern=[[1, SC]],
                                    base=sc * SC - skt * P, channel_multiplier=-1,
                                    compare_op=ALU.is_ge, fill=0.0)
                            nc.tensor.matmul(rs[:], ones_bf[:], ex[:],
                                             start=(skt == 0), stop=(skt == nsk - 1))
                            nc.tensor.matmul(pv[:],
                                             qkvT[NJ - 1][:, skt * P:(skt + 1) * P],
                                             ex[:], start=(skt == 0),
                                             stop=(skt == nsk - 1))
                        rcp = awork.tile([1, SC], F32, tag="rcp", bufs=2)
                        nc.vector.reciprocal(rcp[:], rs[:])
                        rcpb = t2k(tag="rcpb", bufs=3)
                        nc.gpsimd.partition_broadcast(rcpb[:], rcp[:])
                        nc.vector.tensor_tensor(attnT[h][:, cs], pv[:], rcpb[:],
                                                ALU.mult)

                    # ---- o_proj chunk -> ReduceScatter
                    for m in range(NT_HID):
                        wm = wstr.tile([P, HQ, P], BF16, tag="wos", bufs=4)
                        nc.sync.dma_start(wm[:], wo_v[:, :, m * P:(m + 1) * P])
                        ps = acc.tile([P, SC], F32, tag="acc")
                        for a in range(HQ):
                            nc.tensor.matmul(ps[:], wm[:, a, :], attnT[a][:, cs],
                                             start=(a == 0), stop=(a == HQ - 1))
                        ev = t2k()
                        nc.vector.tensor_copy(ev[:], ps[:])
                        nc.sync.dma_start(rs1_in[sc][m * P:(m + 1) * P, :], ev[:])
                    nc.gpsimd.collective_compute(
                        "ReduceScatter", ALU.add, replica_groups=RG,
                        ins=[rs1_in[sc][:].opt()], outs=[rs1_out[sc][:].opt()])

                    # ---- residual2 + rmsnorm2 chunk -> AllGather h2
                    r2 = []
                    ps2 = rowps.tile([1, SC], F32, tag="row", name=f"ssq2_{sc}")
                    for i in range(NT_HSH):
                        o = t2k()
                        nc.sync.dma_start(o[:], rs1_out[sc][i * P:(i + 1) * P, :])
                        xr = t2k()
                        nc.sync.dma_start(xr[:], xT_d[i * P:(i + 1) * P, cs])
                        r2t = awork.tile([P, SC], F32, tag="r2", bufs=NT_HSH + 2)
                        nc.vector.tensor_tensor(r2t[:], o[:], xr[:], ALU.add)
                        nc.sync.dma_start(out_res2[i * P:(i + 1) * P, cs], r2t[:])
                        ssq_accum(ps2, r2t[:], i == 0, i == NT_HSH - 1)
                        r2.append(r2t)
                    ssq2 = awork.tile([1, SC], F32, tag="ssq2", bufs=2)
                    nc.vector.tensor_copy(ssq2[:], ps2[:])
                    nc.sync.dma_start(ar2_in[sc][:], ssq2[:])
                    nc.gpsimd.collective_compute(
                        "AllReduce", ALU.add, replica_groups=RG,
                        ins=[ar2_in[sc][:].opt()], outs=[ar2_out[sc][:].opt()])
                    scale2 = awork.tile([1, SC], F32, tag="scale2", bufs=2)
                    nc.sync.dma_start(scale2[:], ar2_out[sc][:])
                    nc.scalar.activation(scale2[:], scale2[:], AF.Copy,
                                         scale=1.0 / HID, bias=EPS)
                    nc.vector.reciprocal(scale2[:], scale2[:])
                    nc.scalar.activation(scale2[:], scale2[:], AF.Sqrt)
                    scale2b = t2k(tag="scale2b", bufs=2)
                    nc.gpsimd.partition_broadcast(scale2b[:], scale2[:])
                    for i in range(NT_HSH):
                        h2 = t2k()
                        nc.vector.tensor_tensor(h2[:], r2[i][:], scale2b[:], ALU.mult)
                        h2b = t1k()
                        nc.scalar.activation(h2b[:], h2[:], AF.Copy,
                                             scale=ln2_sb[:, i:i + 1])
                        nc.sync.dma_start(ag2_in[sc][i * P:(i + 1) * P, :], h2b[:])
                    nc.gpsimd.collective_compute(
                        "AllGather", ALU.bypass, replica_groups=RG,
                        ins=[ag2_in[sc][:].opt()], outs=[ag2_out[sc][:].opt()])

            # ================== MLP era ==================
            with tc.tile_pool(name="mpersist", bufs=1) as mpersist, \
                 tc.tile_pool(name="mwork", bufs=1) as mwork, \
                 tc.tile_pool(name="mstr", bufs=1) as mstr:
                uT = [mpersist.tile([P, S], BF16, name=f"uT{it}")
                      for it in range(NT_INT)]
                for half in range(2):
                    hs = [half * 2, half * 2 + 1]
                    h2g = []
                    for k in range(NT_HID):
                        g = mwork.tile([P, 2 * SC], BF16, tag=f"h2g{k}", bufs=1)
                        for ci, sc_ in enumerate(hs):
                            nc.sync.dma_start(g[:, ci * SC:(ci + 1) * SC],
                                              ag2_out[sc_][k * P:(k + 1) * P, :])
                        h2g.append(g)
                    for it in range(NT_INT):
                        wt = mstr.tile([P, NT_HID, P], BF16, tag="wups", bufs=2)
                        nc.sync.dma_start(wt[:], wup_v[:, :, it * P:(it + 1) * P])
                        for ci, sc_ in enumerate(hs):
                            ps = acc.tile([P, SC], F32, tag="acc")
                            for k in range(NT_HID):
                                nc.tensor.matmul(ps[:], wt[:, k, :],
                                                 h2g[k][:, ci * SC:(ci + 1) * SC],
                                                 start=(k == 0),
                                                 stop=(k == NT_HID - 1))
                            rl = mwork.tile([P, SC], F32, tag="relu", bufs=3)
                            nc.scalar.activation(rl[:], ps[:], AF.Relu)
                            nc.vector.tensor_tensor(
                                uT[it][:, sc_ * SC:(sc_ + 1) * SC], rl[:], rl[:],
                                ALU.mult)

                for m in range(NT_HID):
                    wdn_t = []
                    for it in range(NT_INT):
                        wf = mstr.tile([P, P], F32, tag="wdnf", bufs=4)
                        nc.sync.dma_start(
                            wf[:], wdn[it * P:(it + 1) * P, m * P:(m + 1) * P])
                        wb = mstr.tile([P, P], BF16, tag="wdnb", bufs=NT_INT + 2)
                        nc.gpsimd.tensor_copy(wb[:], wf[:])
                        wdn_t.append(wb)
                    for sc in range(NSC):
                        ps = acc.tile([P, SC], F32, tag="acc")
                        for it in range(NT_INT):
                            nc.tensor.matmul(ps[:], wdn_t[it][:],
                                             uT[it][:, sc * SC:(sc + 1) * SC],
                                             start=(it == 0), stop=(it == NT_INT - 1))
                        ev = mwork.tile([P, SC], F32, tag="dnev", bufs=3)
                        nc.vector.tensor_copy(ev[:], ps[:])
                        nc.sync.dma_start(rs2_in[sc][m * P:(m + 1) * P, :], ev[:])

                for sc in range(NSC):
                    nc.gpsimd.collective_compute(
                        "ReduceScatter", ALU.add, replica_groups=RG,
                        ins=[rs2_in[sc][:].opt()], outs=[rs2_out[sc][:].opt()])
                    nc.sync.dma_start(out_mlp[:, sc * SC:(sc + 1) * SC], rs2_out[sc][:])

    nc.compile()
    return nc


def shard_inputs(positions, hidden_states, residual, qkv_w, o_w, up_w, down_w,
                 ln1_w, ln2_w):
    hTf = np.ascontiguousarray(np.asarray(hidden_states).reshape(S, HID).T)
    rTf = np.ascontiguousarray(np.asarray(residual).reshape(S, HID).T)
    pos = np.ascontiguousarray(np.asarray(positions).reshape(1, S))
    ln1_t = np.ascontiguousarray(np.asarray(ln1_w).reshape(NT_HID, P).T)  # [128,32]
    ln2_t = np.ascontiguousarray(np.asarray(ln2_w).reshape(NT_HID, P).T)
    q_size = N_HEADS * DHEAD
    kv = N_KV * DHEAD
    in_maps = []
    for c in range(N_CORES):
        wqkv_c = np.concatenate([
            qkv_w[:, c * HQ * DHEAD:(c + 1) * HQ * DHEAD],
            qkv_w[:, q_size + c * DHEAD:q_size + (c + 1) * DHEAD],
            qkv_w[:, q_size + kv + c * DHEAD:q_size + kv + (c + 1) * DHEAD],
        ], axis=1)
        in_maps.append({
            "hT": np.ascontiguousarray(hTf[c * HID_SH:(c + 1) * HID_SH]),
            "rT": np.ascontiguousarray(rTf[c * HID_SH:(c + 1) * HID_SH]),
            "positions": pos,
            "wqkv": np.ascontiguousarray(wqkv_c),
            "wo": np.ascontiguousarray(o_w[c * HQ * DHEAD:(c + 1) * HQ * DHEAD, :]),
            "wup": np.ascontiguousarray(up_w[:, c * INT_SH:(c + 1) * INT_SH]),
            "wdn": np.ascontiguousarray(down_w[c * INT_SH:(c + 1) * INT_SH, :]),
            "ln1": np.ascontiguousarray(ln1_t[:, c * NT_HSH:(c + 1) * NT_HSH]),
            "ln2": np.ascontiguousarray(ln2_t[:, c * NT_HSH:(c + 1) * NT_HSH]),
        })
    return in_maps


_CACHE = {}


def kernel(**inputs):
    from concourse.bass_utils import run_bass_kernel_spmd
    if "nc" not in _CACHE:
        _CACHE["nc"] = build_graph()
    nc = _CACHE["nc"]
    in_maps = shard_inputs(**{k: np.asarray(v) for k, v in inputs.items()})
    res = run_bass_kernel_spmd(nc, in_maps, core_ids=list(range(N_CORES)),
                               trace=False)
    mlpT = np.concatenate([res.results[c]["mlpT"] for c in range(N_CORES)], axis=0)
    res2T = np.concatenate([res.results[c]["res2T"] for c in range(N_CORES)], axis=0)
    mlp_out = np.ascontiguousarray(mlpT.T).reshape(1, S, HID)
    residual2 = np.ascontiguousarray(res2T.T).reshape(1, S, HID)
    return mlp_out, residual2


# revision 11
# speedup vs baseline: 1.0471x; 1.0471x over previous
"""Arcee decoder layer on 8 TRN2 NeuronCores — tensor-parallel Bass kernel.

Sharding (8-way TP, transposed activation layout [hidden, seq] on device):
  - core c owns: q heads 4c..4c+3 + kv head c (GQA group), residual-stream
    rows 512c..512c+512, intermediate cols 2048c..2048c+2048.
  - RMSNorm trick: the un-normalized residual stream is AllGathered (bf16)
    with each core's partial sum-of-squares embedded as an extra row; every
    core derives the per-token rsqrt scale locally and the scale is folded
    into the next matmul's PSUM eviction (norm scale commutes with the
    matmul). ln weights are folded into the bf16 weight caches.
  - o_proj / down_proj emit transposed partials [4096,S]; bf16 ReduceScatter
    gives each core its hid-slice of the sum = its slice of the transposed
    outputs. Host reassembles by concat + transpose.
  - dtypes: all matmuls bf16 (f32 PSUM accumulation); residual stream and
    softmax statistics f32.
"""
import sys

sys.path.insert(0, "/opt/trn_rl_repo")

import math
import numpy as np

import concourse.bass as bass
import concourse.mybir as mybir
import concourse.tile as tile
from concourse import bacc
from concourse.bass_isa import ReduceOp
from concourse.masks import make_identity

F32 = mybir.dt.float32
BF16 = mybir.dt.bfloat16
I32 = mybir.dt.int32
AF = mybir.ActivationFunctionType
ALU = mybir.AluOpType

N_CORES = 8
S = 2048
HID = 4096
N_HEADS = 32
N_KV = 8
DHEAD = 128
INTER = 16384
EPS = 1e-5
THETA = 10000.0

HQ = N_HEADS // N_CORES          # 4 q heads per core
HID_SH = HID // N_CORES          # 512 residual rows per core
INT_SH = INTER // N_CORES        # 2048 intermediate per core
NJ = HQ + 2                      # qkv col tiles per core (4q + k + v)
QKV_COLS = NJ * DHEAD            # 768
P = 128
SC = 512                         # seq chunk (matmul moving dim)
NSC = S // SC                    # 4
NT_HID = HID // P                # 32
NT_HSH = HID_SH // P             # 4
NT_INT = INT_SH // P             # 16
BLK = HID_SH + 8                 # AG block rows: 512 payload + ssq row + pad
TWO_PI = 2.0 * math.pi


def build_graph():
    nc = bacc.Bacc(None, target_bir_lowering=False, debug=False)

    hT = nc.declare_dram_parameter("hT", [HID_SH, S], F32, isOutput=False)
    rT = nc.declare_dram_parameter("rT", [HID_SH, S], F32, isOutput=False)
    pos_in = nc.declare_dram_parameter("positions", [1, S], I32, isOutput=False)
    wqkv = nc.declare_dram_parameter("wqkv", [HID, QKV_COLS], F32, isOutput=False)
    wo = nc.declare_dram_parameter("wo", [HQ * DHEAD, HID], F32, isOutput=False)
    wup = nc.declare_dram_parameter("wup", [HID, INT_SH], F32, isOutput=False)
    wdn = nc.declare_dram_parameter("wdn", [INT_SH, HID], F32, isOutput=False)
    ln1 = nc.declare_dram_parameter("ln1", [P, NT_HID], F32, isOutput=False)
    ln2 = nc.declare_dram_parameter("ln2", [P, NT_HID], F32, isOutput=False)
    out_res2 = nc.declare_dram_parameter("res2T", [HID_SH, S], F32, isOutput=True)
    out_mlp = nc.declare_dram_parameter("mlpT", [HID_SH, S], F32, isOutput=True)

    RG = [list(range(N_CORES))]
    inv_sqrt_d = 1.0 / math.sqrt(DHEAD)

    with tile.TileContext(nc) as tc:
        import contextlib
        with contextlib.ExitStack() as ctx:
            const = ctx.enter_context(tc.tile_pool(name="const", bufs=1))
            rowsb = ctx.enter_context(tc.tile_pool(name="rowsb", bufs=1))
            acc = ctx.enter_context(tc.tile_pool(name="acc", bufs=5, space="PSUM"))
            rowps = ctx.enter_context(tc.tile_pool(name="rowps", bufs=2, space="PSUM"))
            tpps = ctx.enter_context(tc.tile_pool(name="tpps", bufs=1, space="PSUM"))
            dram = ctx.enter_context(tc.tile_pool(name="dram", bufs=1, space="DRAM"))

            # ============ constants ============
            ident = const.tile([P, P], BF16)
            make_identity(nc, ident[:])
            ones_bf = const.tile([P, 1], BF16)
            nc.vector.memset(ones_bf[:], 1.0)
            ln1_sb = const.tile([P, NT_HID], F32)
            ln2_sb = const.tile([P, NT_HID], F32)
            nc.sync.dma_start(ln1_sb[:], ln1[:])
            nc.sync.dma_start(ln2_sb[:], ln2[:])
            cos2 = const.tile([P, S], BF16)
            sin_neg = const.tile([P, S], BF16)

            # DRAM scratch
            wqkv_c = dram.tile([P, NT_HID * QKV_COLS], BF16, name="wqkv_c")
            wo_c = dram.tile([P, HQ * HID], BF16, name="wo_c")
            wup_c = dram.tile([P, NT_HID * INT_SH], BF16, name="wup_c")
            xT_d = dram.tile([HID_SH, S], F32, name="xT_d")
            ag1_in = dram.tile([BLK, S], BF16, name="ag1_in")
            ag1_out = dram.tile([N_CORES * BLK, S], BF16, name="ag1_out",
                                addr_space="Shared")
            ag2_in = dram.tile([BLK, S], BF16, name="ag2_in")
            ag2_out = dram.tile([N_CORES * BLK, S], BF16, name="ag2_out",
                                addr_space="Shared")
            rs1_in = [dram.tile([HID, SC], BF16, name=f"rs1_in{sc}") for sc in range(NSC)]
            rs1_out = [dram.tile([HID_SH, SC], BF16, name=f"rs1_out{sc}")
                       for sc in range(NSC)]
            rs2_in = dram.tile([HID, S], BF16, name="rs2_in")
            rs2_out = dram.tile([HID_SH, S], BF16, name="rs2_out")

            ag1_v = ag1_out[:].rearrange("(c r) s -> c r s", r=BLK)
            ag2_v = ag2_out[:].rearrange("(c r) s -> c r s", r=BLK)
            wqkv_v = wqkv_c[:].rearrange("p (k c) -> p k c", k=NT_HID)
            wo_v = wo_c[:].rearrange("p (a c) -> p a c", a=HQ)
            wup_v = wup_c[:].rearrange("p (k c) -> p k c", k=NT_HID)

            def hg_src(ag_v, k, cs):
                # global hid tile k of the gathered (blocked) activation
                return ag_v[k // NT_HSH, (k % NT_HSH) * P:(k % NT_HSH + 1) * P, cs]

            # ============ rope tables (scoped scratch) ============
            with tc.tile_pool(name="tbl", bufs=1) as tbl:
                iot = tbl.tile([64, 1], I32)
                nc.gpsimd.iota(iot[:], pattern=[[1, 1]], base=0, channel_multiplier=1)
                iotf = tbl.tile([64, 1], F32)
                nc.vector.tensor_copy(iotf[:], iot[:])
                invf = tbl.tile([64, 1], F32)
                nc.scalar.activation(invf[:], iotf[:], AF.Exp,
                                     scale=-math.log(THETA) / 64.0)
                invf2pi = tbl.tile([64, 1], F32)
                nc.scalar.activation(invf2pi[:], invf[:], AF.Copy,
                                     scale=1.0 / TWO_PI)
                posi = tbl.tile([1, S], I32)
                nc.sync.dma_start(posi[:], pos_in[:])
                posf = tbl.tile([1, S], F32)
                nc.vector.tensor_copy(posf[:], posi[:])
                posb = tbl.tile([64, S], F32)
                nc.gpsimd.partition_broadcast(posb[:], posf[:])

                def range_reduce_sin(dst_bf, t_ap, negate=False):
                    # dst = sin(2*pi*t) via two-stage round-and-subtract
                    n1 = tbl.tile([64, S], I32, tag="rri", bufs=2, name="n1")
                    nc.vector.tensor_copy(n1[:], t_ap)
                    n1f = tbl.tile([64, S], F32, tag="rrf", bufs=2, name="n1f")
                    nc.vector.tensor_copy(n1f[:], n1[:])
                    f1 = tbl.tile([64, S], F32, tag="rrg", bufs=2, name="f1")
                    nc.vector.tensor_tensor(f1[:], t_ap, n1f[:], ALU.subtract)
                    n2 = tbl.tile([64, S], I32, tag="rri", bufs=2, name="n2")
                    nc.vector.tensor_copy(n2[:], f1[:])
                    n2f = tbl.tile([64, S], F32, tag="rrf", bufs=2, name="n2f")
                    nc.vector.tensor_copy(n2f[:], n2[:])
                    f2 = tbl.tile([64, S], F32, tag="rrg", bufs=2, name="f2")
                    nc.vector.tensor_tensor(f2[:], f1[:], n2f[:], ALU.subtract)
                    nc.scalar.activation(dst_bf, f2[:], AF.Sin,
                                         scale=-TWO_PI if negate else TWO_PI)

                tfrac = tbl.tile([64, S], F32)
                nc.scalar.activation(tfrac[:], posb[:], AF.Copy, scale=invf2pi[:])
                sinb = tbl.tile([64, S], BF16)
                sinnb = tbl.tile([64, S], BF16)
                range_reduce_sin(sinb[:], tfrac[:])
                range_reduce_sin(sinnb[:], tfrac[:], negate=True)
                tfrac2 = tbl.tile([64, S], F32)
                nc.scalar.activation(tfrac2[:], tfrac[:], AF.Copy, bias=0.25)
                cosb = tbl.tile([64, S], BF16)
                range_reduce_sin(cosb[:], tfrac2[:])
                nc.sync.dma_start(cos2[:64, :], cosb[:])
                nc.sync.dma_start(cos2[64:, :], cosb[:])
                nc.sync.dma_start(sin_neg[:64, :], sinnb[:])
                nc.sync.dma_start(sin_neg[64:, :], sinb[:])

            # ================== attention era ==================
            with tc.tile_pool(name="apersist", bufs=1) as apersist, \
                 tc.tile_pool(name="awork", bufs=1) as awork, \
                 tc.tile_pool(name="wstr", bufs=1) as wstr:

                _cnt = [0]

                def t2k(tag="t2k", bufs=9):
                    _cnt[0] += 1
                    return awork.tile([P, SC], F32, tag=tag, bufs=bufs,
                                      name=f"t_{_cnt[0]}")

                def t1k(tag="t1k", bufs=8):
                    _cnt[0] += 1
                    return awork.tile([P, SC], BF16, tag=tag, bufs=bufs,
                                      name=f"t_{_cnt[0]}")

                # ---- phase 1: x = h + r (chunked), ssq row, ag1_in (bf16)
                ssq1 = awork.tile([1, S], F32, name="ssq1")
                for sc in range(NSC):
                    cs = slice(sc * SC, (sc + 1) * SC)
                    ps = rowps.tile([1, SC], F32, tag="row", name=f"ssq1p{sc}")
                    for i in range(NT_HSH):
                        a = t2k()
                        b = t2k()
                        nc.sync.dma_start(a[:], hT[i * P:(i + 1) * P, cs])
                        nc.sync.dma_start(b[:], rT[i * P:(i + 1) * P, cs])
                        xt = t2k()
                        nc.vector.tensor_tensor(xt[:], a[:], b[:], ALU.add)
                        nc.sync.dma_start(xT_d[i * P:(i + 1) * P, cs], xt[:])
                        xb = t1k()
                        nc.vector.tensor_copy(xb[:], xt[:])
                        nc.sync.dma_start(ag1_in[i * P:(i + 1) * P, cs], xb[:])
                        sq = t1k(tag="sq", bufs=4)
                        nc.scalar.activation(sq[:], xt[:], AF.Square)
                        nc.tensor.matmul(ps[:], ones_bf[:], sq[:],
                                         start=(i == 0), stop=(i == NT_HSH - 1))
                    nc.vector.tensor_copy(ssq1[:, cs], ps[:])
                ssq1b = awork.tile([1, S], BF16, name="ssq1b")
                nc.vector.tensor_copy(ssq1b[:], ssq1[:])
                nc.sync.dma_start(ag1_in[HID_SH:HID_SH + 1, :], ssq1b[:])
                nc.gpsimd.collective_compute("AllGather", ALU.bypass, replica_groups=RG,
                                             ins=[ag1_in[:].opt()],
                                             outs=[ag1_out[:].opt()])

                # ---- weight caches (emitted after AG1 so phase-1 DMAs win)
                def build_cache(src, n_row_tiles, n_cols, dst, ln_col, eng):
                    CB = min(n_cols, 1024)
                    for k in range(n_row_tiles):
                        for c0 in range(0, n_cols, CB):
                            wf = wstr.tile([P, CB], F32, tag="cbf", bufs=2,
                                           name=f"cb_{dst.tensor.name}_{k}_{c0}")
                            nc.sync.dma_start(wf[:, :min(CB, n_cols - c0)],
                                              src[k * P:(k + 1) * P,
                                                  c0:c0 + min(CB, n_cols - c0)])
                            wb = wstr.tile([P, CB], BF16, tag="cbb", bufs=2,
                                           name=f"cc_{dst.tensor.name}_{k}_{c0}")
                            w = min(CB, n_cols - c0)
                            if ln_col is not None:
                                nc.scalar.activation(wb[:, :w], wf[:, :w], AF.Copy,
                                                     scale=ln_col[:, k:k + 1])
                            else:
                                eng.tensor_copy(wb[:, :w], wf[:, :w])
                            nc.sync.dma_start(
                                dst[:, k * n_cols + c0:k * n_cols + c0 + w],
                                wb[:, :w])

                build_cache(wqkv, NT_HID, QKV_COLS, wqkv_c, ln1_sb, None)
                build_cache(wo, HQ, HID, wo_c, None, nc.gpsimd)
                build_cache(wup, NT_HID, INT_SH, wup_c, ln2_sb, None)

                # per-chunk norm1 scale rows (from gathered ssq partials)
                s1b = apersist.tile([P, S], F32, name="s1b")  # rsqrt scale bcast
                for sc in range(NSC):
                    cs = slice(sc * SC, (sc + 1) * SC)
                    srows_b = awork.tile([8, SC], BF16, tag="srb", bufs=1,
                                         name=f"sr1b{sc}")
                    nc.sync.dma_start(srows_b[:], ag1_v[:, HID_SH, cs])
                    srows = awork.tile([8, SC], F32, tag="srf", bufs=1,
                                       name=f"sr1f{sc}")
                    nc.vector.tensor_copy(srows[:], srows_b[:])
                    ssum = awork.tile([8, SC], F32, tag="ssum", bufs=1,
                                      name=f"ss1{sc}")
                    nc.gpsimd.partition_all_reduce(ssum[:], srows[:], channels=8,
                                                   reduce_op=ReduceOp.add)
                    var = awork.tile([1, SC], F32, tag="var", bufs=2, name=f"v1{sc}")
                    nc.scalar.activation(var[:], ssum[:1, :], AF.Copy,
                                         scale=1.0 / HID, bias=EPS)
                    nc.vector.reciprocal(var[:], var[:])
                    nc.scalar.activation(var[:], var[:], AF.Sqrt)
                    nc.gpsimd.partition_broadcast(s1b[:, cs], var[:])

                qkvT = [apersist.tile([P, S], BF16, name=f"qkvT{j}") for j in range(NJ)]
                attnT = [apersist.tile([P, S], BF16, name=f"attnT{a}") for a in range(HQ)]

                for sc in range(NSC):
                    cs = slice(sc * SC, (sc + 1) * SC)
                    # ---- qkv chunk (scale1 folded into eviction)
                    hg = []
                    for k in range(NT_HID):
                        g = awork.tile([P, SC], BF16, tag=f"hg{k}", bufs=1,
                                       name=f"hg{k}_{sc}")
                        nc.sync.dma_start(g[:], hg_src(ag1_v, k, cs))
                        hg.append(g)
                    for j in range(NJ):
                        KH = NT_HID // 2
                        wj0 = wstr.tile([P, KH, P], BF16, tag="wqkvs", bufs=3,
                                        name=f"wj{j}_{sc}a")
                        nc.sync.dma_start(wj0[:], wqkv_v[:, :KH, j * P:(j + 1) * P])
                        wj1 = wstr.tile([P, KH, P], BF16, tag="wqkvs", bufs=3,
                                        name=f"wj{j}_{sc}b")
                        nc.sync.dma_start(wj1[:], wqkv_v[:, KH:, j * P:(j + 1) * P])
                        ps = acc.tile([P, SC], F32, tag="acc", name=f"qk{j}_{sc}")
                        for k in range(NT_HID):
                            wk = wj0[:, k, :] if k < KH else wj1[:, k - KH, :]
                            nc.tensor.matmul(ps[:], wk, hg[k][:],
                                             start=(k == 0), stop=(k == NT_HID - 1))
                        nc.vector.tensor_tensor(qkvT[j][:, cs], ps[:], s1b[:, cs],
                                                ALU.mult)

                    # ---- rope on q tiles and k tile (bf16, chunk cols)
                    for j in range(HQ + 1):
                        t = qkvT[j]
                        swp = t1k()
                        nc.sync.dma_start(swp[:64, :], t[64:, cs])
                        nc.sync.dma_start(swp[64:, :], t[:64, cs])
                        m1 = t1k()
                        nc.vector.tensor_tensor(m1[:], t[:, cs], cos2[:, cs], ALU.mult)
                        m2 = t1k()
                        nc.vector.tensor_tensor(m2[:], swp[:], sin_neg[:, cs], ALU.mult)
                        nc.vector.tensor_tensor(t[:, cs], m1[:], m2[:], ALU.add)

                    # ---- v transpose in place (block-transposed v)
                    for t in range(sc * (SC // P), (sc + 1) * (SC // P)):
                        pst = tpps.tile([P, P], BF16, tag="tp", name=f"tp{t}")
                        nc.tensor.transpose(pst[:], qkvT[NJ - 1][:, t * P:(t + 1) * P],
                                            ident[:])
                        nc.vector.tensor_copy(qkvT[NJ - 1][:, t * P:(t + 1) * P], pst[:])

                    # ---- attention (4 heads x this chunk)
                    nsk = (sc + 1) * (SC // P)
                    for h in range(HQ):
                        pv = acc.tile([P, SC], F32, tag="acc", name=f"pv{h}_{sc}")
                        rs = rowps.tile([1, SC], F32, tag="row", name=f"rs{h}_{sc}")
                        for skt in range(nsk):
                            sps = acc.tile([P, SC], F32, tag="acc",
                                           name=f"s{h}_{sc}_{skt}")
                            nc.tensor.matmul(sps[:],
                                             qkvT[HQ][:, skt * P:(skt + 1) * P],
                                             qkvT[h][:, cs], start=True, stop=True)
                            ex = t1k(tag="ex", bufs=5)
                            nc.scalar.activation(ex[:], sps[:], AF.Exp,
                                                 scale=inv_sqrt_d)
                            if skt >= 4 * sc:
                                nc.gpsimd.affine_select(
                                    ex[:], ex[:], pattern=[[1, SC]],
                                    base=sc * SC - skt * P, channel_multiplier=-1,
                                    compare_op=ALU.is_ge, fill=0.0)
                            nc.tensor.matmul(rs[:], ones_bf[:], ex[:],
                                             start=(skt == 0), stop=(skt == nsk - 1))
                            nc.tensor.matmul(pv[:],
                                             qkvT[NJ - 1][:, skt * P:(skt + 1) * P],
                                             ex[:], start=(skt == 0),
                                             stop=(skt == nsk - 1))
                        rcp = awork.tile([1, SC], F32, tag="rcp", bufs=2,
                                         name=f"rcp{h}_{sc}")
                        nc.vector.reciprocal(rcp[:], rs[:])
                        rcpb = t2k(tag="rcpb", bufs=2)
                        nc.gpsimd.partition_broadcast(rcpb[:], rcp[:])
                        nc.vector.tensor_tensor(attnT[h][:, cs], pv[:], rcpb[:],
                                                ALU.mult)

                    # ---- o_proj chunk -> bf16 ReduceScatter
                    for m in range(NT_HID):
                        wm = wstr.tile([P, HQ, P], BF16, tag="wos", bufs=2,
                                       name=f"wm{m}_{sc}")
                        nc.sync.dma_start(wm[:], wo_v[:, :, m * P:(m + 1) * P])
                        ps = acc.tile([P, SC], F32, tag="acc", name=f"o{m}_{sc}")
                        for a in range(HQ):
                            nc.tensor.matmul(ps[:], wm[:, a, :], attnT[a][:, cs],
                                             start=(a == 0), stop=(a == HQ - 1))
                        ev = t1k(tag="oev", bufs=3)
                        nc.vector.tensor_copy(ev[:], ps[:])
                        nc.sync.dma_start(rs1_in[sc][m * P:(m + 1) * P, :], ev[:])
                    nc.gpsimd.collective_compute(
                        "ReduceScatter", ALU.add, replica_groups=RG,
                        ins=[rs1_in[sc][:].opt()], outs=[rs1_out[sc][:].opt()])

                    # ---- residual2 chunk -> raw bf16 + ssq row into ag2_in
                    ps2 = rowps.tile([1, SC], F32, tag="row", name=f"ssq2_{sc}")
                    for i in range(NT_HSH):
                        o = t1k(tag="r2ld", bufs=3)
                        nc.sync.dma_start(o[:], rs1_out[sc][i * P:(i + 1) * P, :])
                        xr = t2k()
                        nc.sync.dma_start(xr[:], xT_d[i * P:(i + 1) * P, cs])
                        r2t = t2k(tag="r2", bufs=5)
                        nc.vector.tensor_tensor(r2t[:], o[:], xr[:], ALU.add)
                        nc.sync.dma_start(out_res2[i * P:(i + 1) * P, cs], r2t[:])
                        r2b = t1k()
                        nc.vector.tensor_copy(r2b[:], r2t[:])
                        nc.sync.dma_start(ag2_in[i * P:(i + 1) * P, cs], r2b[:])
                        sq = t1k(tag="sq", bufs=4)
                        nc.scalar.activation(sq[:], r2t[:], AF.Square)
                        nc.tensor.matmul(ps2[:], ones_bf[:], sq[:],
                                         start=(i == 0), stop=(i == NT_HSH - 1))
                    ssq2 = awork.tile([1, SC], BF16, tag="ssq2", bufs=2,
                                      name=f"sq2_{sc}")
                    nc.vector.tensor_copy(ssq2[:], ps2[:])
                    nc.sync.dma_start(ag2_in[HID_SH:HID_SH + 1, cs], ssq2[:])

                nc.gpsimd.collective_compute("AllGather", ALU.bypass, replica_groups=RG,
                                             ins=[ag2_in[:].opt()],
                                             outs=[ag2_out[:].opt()])

            # ================== MLP era ==================
            with tc.tile_pool(name="mpersist", bufs=1) as mpersist, \
                 tc.tile_pool(name="mwork", bufs=1) as mwork, \
                 tc.tile_pool(name="mstr", bufs=1) as mstr:
                # per-chunk 1/var rows (scale2^2 fold for down eviction)
                s2b = mpersist.tile([P, S], F32, name="s2b")
                for sc in range(NSC):
                    cs = slice(sc * SC, (sc + 1) * SC)
                    srows_b = mwork.tile([8, SC], BF16, tag="srb", bufs=1,
                                         name=f"sr2b{sc}")
                    nc.sync.dma_start(srows_b[:], ag2_v[:, HID_SH, cs])
                    srows = mwork.tile([8, SC], F32, tag="srf", bufs=1,
                                       name=f"sr2f{sc}")
                    nc.vector.tensor_copy(srows[:], srows_b[:])
                    ssum = mwork.tile([8, SC], F32, tag="ssum", bufs=1,
                                      name=f"ss2{sc}")
                    nc.gpsimd.partition_all_reduce(ssum[:], srows[:], channels=8,
                                                   reduce_op=ReduceOp.add)
                    var = mwork.tile([1, SC], F32, tag="var", bufs=2, name=f"v2{sc}")
                    nc.scalar.activation(var[:], ssum[:1, :], AF.Copy,
                                         scale=1.0 / HID, bias=EPS)
                    nc.vector.reciprocal(var[:], var[:])  # = scale2^2
                    nc.gpsimd.partition_broadcast(s2b[:, cs], var[:])

                uT = [mpersist.tile([P, S], BF16, name=f"uT{it}")
                      for it in range(NT_INT)]
                for half in range(2):
                    hs = [half * 2, half * 2 + 1]
                    h2g = []
                    for k in range(NT_HID):
                        g = mwork.tile([P, 2 * SC], BF16, tag=f"h2g{k}", bufs=1,
                                       name=f"h2g{k}_{half}")
                        for ci, sc_ in enumerate(hs):
                            nc.sync.dma_start(
                                g[:, ci * SC:(ci + 1) * SC],
                                hg_src(ag2_v, k, slice(sc_ * SC, (sc_ + 1) * SC)))
                        h2g.append(g)
                    for it in range(NT_INT):
                        KH = NT_HID // 2
                        wt0 = mstr.tile([P, KH, P], BF16, tag="wups", bufs=3,
                                        name=f"wt{it}_{half}a")
                        nc.sync.dma_start(wt0[:], wup_v[:, :KH, it * P:(it + 1) * P])
                        wt1 = mstr.tile([P, KH, P], BF16, tag="wups", bufs=3,
                                        name=f"wt{it}_{half}b")
                        nc.sync.dma_start(wt1[:], wup_v[:, KH:, it * P:(it + 1) * P])
                        for ci, sc_ in enumerate(hs):
                            ps = acc.tile([P, SC], F32, tag="acc",
                                          name=f"up{it}_{sc_}")
                            for k in range(NT_HID):
                                wk = wt0[:, k, :] if k < KH else wt1[:, k - KH, :]
                                nc.tensor.matmul(ps[:], wk,
                                                 h2g[k][:, ci * SC:(ci + 1) * SC],
                                                 start=(k == 0),
                                                 stop=(k == NT_HID - 1))
                            rl = mwork.tile([P, SC], F32, tag="relu", bufs=2,
                                            name=f"rl{it}_{sc_}")
                            nc.scalar.activation(rl[:], ps[:], AF.Relu)
                            nc.vector.tensor_tensor(
                                uT[it][:, sc_ * SC:(sc_ + 1) * SC], rl[:], rl[:],
                                ALU.mult)

                for m in range(NT_HID):
                    wdn_t = []
                    for it in range(NT_INT):
                        wf = mstr.tile([P, P], F32, tag="wdnf", bufs=4,
                                       name=f"wf{m}_{it}")
                        nc.sync.dma_start(
                            wf[:], wdn[it * P:(it + 1) * P, m * P:(m + 1) * P])
                        wb = mstr.tile([P, P], BF16, tag="wdnb", bufs=NT_INT + 2,
                                       name=f"wb{m}_{it}")
                        nc.gpsimd.tensor_copy(wb[:], wf[:])
                        wdn_t.append(wb)
                    for sc in range(NSC):
                        cs = slice(sc * SC, (sc + 1) * SC)
                        ps = acc.tile([P, SC], F32, tag="acc", name=f"dn{m}_{sc}")
                        for it in range(NT_INT):
                            nc.tensor.matmul(ps[:], wdn_t[it][:], uT[it][:, cs],
                                             start=(it == 0), stop=(it == NT_INT - 1))
                        ev = mwork.tile([P, SC], BF16, tag="dnev", bufs=3,
                                        name=f"dev{m}_{sc}")
                        nc.vector.tensor_tensor(ev[:], ps[:], s2b[:, cs], ALU.mult)
                        nc.sync.dma_start(rs2_in[m * P:(m + 1) * P, cs], ev[:])

                nc.gpsimd.collective_compute(
                    "ReduceScatter", ALU.add, replica_groups=RG,
                    ins=[rs2_in[:].opt()], outs=[rs2_out[:].opt()])
                for i in range(NT_HSH):
                    for sc in range(NSC):
                        cs = slice(sc * SC, (sc + 1) * SC)
                        mo = mwork.tile([P, SC], BF16, tag="mold", bufs=2,
                                        name=f"mo{i}_{sc}")
                        nc.sync.dma_start(mo[:], rs2_out[i * P:(i + 1) * P, cs])
                        mof = mwork.tile([P, SC], F32, tag="mof", bufs=2,
                                         name=f"mf{i}_{sc}")
                        nc.vector.tensor_copy(mof[:], mo[:])
                        nc.sync.dma_start(out_mlp[i * P:(i + 1) * P, cs], mof[:])

    nc.compile()
    return nc


def shard_inputs(positions, hidden_states, residual, qkv_w, o_w, up_w, down_w,
                 ln1_w, ln2_w):
    hTf = np.ascontiguousarray(np.asarray(hidden_states).reshape(S, HID).T)
    rTf = np.ascontiguousarray(np.asarray(residual).reshape(S, HID).T)
    pos = np.ascontiguousarray(np.asarray(positions).reshape(1, S))
    ln1_t = np.ascontiguousarray(np.asarray(ln1_w).reshape(NT_HID, P).T)  # [128,32]
    ln2_t = np.ascontiguousarray(np.asarray(ln2_w).reshape(NT_HID, P).T)
    q_size = N_HEADS * DHEAD
    kv = N_KV * DHEAD
    in_maps = []
    for c in range(N_CORES):
        wqkv_c = np.concatenate([
            qkv_w[:, c * HQ * DHEAD:(c + 1) * HQ * DHEAD],
            qkv_w[:, q_size + c * DHEAD:q_size + (c + 1) * DHEAD],
            qkv_w[:, q_size + kv + c * DHEAD:q_size + kv + (c + 1) * DHEAD],
        ], axis=1)
        in_maps.append({
            "hT": np.ascontiguousarray(hTf[c * HID_SH:(c + 1) * HID_SH]),
            "rT": np.ascontiguousarray(rTf[c * HID_SH:(c + 1) * HID_SH]),
            "positions": pos,
            "wqkv": np.ascontiguousarray(wqkv_c),
            "wo": np.ascontiguousarray(o_w[c * HQ * DHEAD:(c + 1) * HQ * DHEAD, :]),
            "wup": np.ascontiguousarray(up_w[:, c * INT_SH:(c + 1) * INT_SH]),
            "wdn": np.ascontiguousarray(down_w[c * INT_SH:(c + 1) * INT_SH, :]),
            "ln1": ln1_t,
            "ln2": ln2_t,
        })
    return in_maps


_CACHE = {}


def kernel(**inputs):
    from concourse.bass_utils import run_bass_kernel_spmd
    if "nc" not in _CACHE:
        _CACHE["nc"] = build_graph()
    nc = _CACHE["nc"]
    in_maps = shard_inputs(**{k: np.asarray(v) for k, v in inputs.items()})
    res = run_bass_kernel_spmd(nc, in_maps, core_ids=list(range(N_CORES)),
                               trace=False)
    mlpT = np.concatenate([res.results[c]["mlpT"] for c in range(N_CORES)], axis=0)
    res2T = np.concatenate([res.results[c]["res2T"] for c in range(N_CORES)], axis=0)
    mlp_out = np.ascontiguousarray(mlpT.T).reshape(1, S, HID)
    residual2 = np.ascontiguousarray(res2T.T).reshape(1, S, HID)
    return mlp_out, residual2


# revision 13
# speedup vs baseline: 1.1201x; 1.0698x over previous
"""Arcee decoder layer on 8 TRN2 NeuronCores — tensor-parallel Bass kernel.

Sharding (8-way TP, transposed activation layout [hidden, seq] on device):
  - core c owns: q heads 4c..4c+3 + kv head c (GQA group), residual-stream
    rows 512c..512c+512, intermediate cols 2048c..2048c+2048.
  - RMSNorm trick: the un-normalized residual stream is AllGathered (bf16)
    with each core's partial sum-of-squares embedded as an extra row; every
    core derives the per-token rsqrt scale locally and the scale is folded
    into the next matmul's PSUM eviction (norm scale commutes with the
    matmul). ln weights are folded into the bf16 weight caches.
  - o_proj / down_proj emit transposed partials [4096,S]; bf16 ReduceScatter
    gives each core its hid-slice of the sum = its slice of the transposed
    outputs. Host reassembles by concat + transpose.
  - dtypes: all matmuls bf16 (f32 PSUM accumulation); residual stream and
    softmax statistics f32.
"""
import sys

sys.path.insert(0, "/opt/trn_rl_repo")

import math
import numpy as np

import concourse.bass as bass
import concourse.mybir as mybir
import concourse.tile as tile
from concourse import bacc
from concourse.bass_isa import ReduceOp
from concourse.masks import make_identity

F32 = mybir.dt.float32
BF16 = mybir.dt.bfloat16
I32 = mybir.dt.int32
AF = mybir.ActivationFunctionType
ALU = mybir.AluOpType

N_CORES = 8
S = 2048
HID = 4096
N_HEADS = 32
N_KV = 8
DHEAD = 128
INTER = 16384
EPS = 1e-5
THETA = 10000.0

HQ = N_HEADS // N_CORES          # 4 q heads per core
HID_SH = HID // N_CORES          # 512 residual rows per core
INT_SH = INTER // N_CORES        # 2048 intermediate per core
NJ = HQ + 2                      # qkv col tiles per core (4q + k + v)
QKV_COLS = NJ * DHEAD            # 768
P = 128
SC = 512                         # seq chunk (matmul moving dim)
NSC = S // SC                    # 4
NT_HID = HID // P                # 32
NT_HSH = HID_SH // P             # 4
NT_INT = INT_SH // P             # 16
BLK = HID_SH + 8                 # AG block rows: 512 payload + ssq row + pad
TWO_PI = 2.0 * math.pi


def build_graph():
    nc = bacc.Bacc(None, target_bir_lowering=False, debug=False)

    hT = nc.declare_dram_parameter("hT", [HID_SH, S], F32, isOutput=False)
    rT = nc.declare_dram_parameter("rT", [HID_SH, S], F32, isOutput=False)
    pos_in = nc.declare_dram_parameter("positions", [1, S], I32, isOutput=False)
    wqkv = nc.declare_dram_parameter("wqkv", [HID, QKV_COLS], F32, isOutput=False)
    wo = nc.declare_dram_parameter("wo", [HQ * DHEAD, HID], F32, isOutput=False)
    wup = nc.declare_dram_parameter("wup", [HID, INT_SH], F32, isOutput=False)
    wdn = nc.declare_dram_parameter("wdn", [INT_SH, HID], F32, isOutput=False)
    ln1 = nc.declare_dram_parameter("ln1", [P, NT_HID], F32, isOutput=False)
    ln2 = nc.declare_dram_parameter("ln2", [P, NT_HID], F32, isOutput=False)
    out_res2 = nc.declare_dram_parameter("res2T", [HID_SH, S], F32, isOutput=True)
    out_mlp = nc.declare_dram_parameter("mlpT", [HID_SH, S], F32, isOutput=True)

    RG = [list(range(N_CORES))]
    inv_sqrt_d = 1.0 / math.sqrt(DHEAD)

    with tile.TileContext(nc) as tc:
        import contextlib
        with contextlib.ExitStack() as ctx:
            const = ctx.enter_context(tc.tile_pool(name="const", bufs=1))
            rowsb = ctx.enter_context(tc.tile_pool(name="rowsb", bufs=1))
            acc = ctx.enter_context(tc.tile_pool(name="acc", bufs=5, space="PSUM"))
            rowps = ctx.enter_context(tc.tile_pool(name="rowps", bufs=2, space="PSUM"))
            tpps = ctx.enter_context(tc.tile_pool(name="tpps", bufs=1, space="PSUM"))
            dram = ctx.enter_context(tc.tile_pool(name="dram", bufs=1, space="DRAM"))

            # ============ constants ============
            ident = const.tile([P, P], BF16)
            make_identity(nc, ident[:])
            ones_bf = const.tile([P, 1], BF16)
            nc.vector.memset(ones_bf[:], 1.0)
            ln1_sb = const.tile([P, NT_HID], F32)
            ln2_sb = const.tile([P, NT_HID], F32)
            nc.sync.dma_start(ln1_sb[:], ln1[:])
            nc.sync.dma_start(ln2_sb[:], ln2[:])
            cos2 = const.tile([P, S], BF16)
            sin_neg = const.tile([P, S], BF16)

            # DRAM scratch
            wqkv_c = dram.tile([P, NT_HID * QKV_COLS], BF16, name="wqkv_c")
            wo_c = dram.tile([P, HQ * HID], BF16, name="wo_c")
            wup_c = dram.tile([P, NT_HID * INT_SH], BF16, name="wup_c")
            xT_d = dram.tile([HID_SH, S], F32, name="xT_d")
            ag1_in = dram.tile([BLK, S], BF16, name="ag1_in")
            ag1_out = dram.tile([N_CORES * BLK, S], BF16, name="ag1_out",
                                addr_space="Shared")
            ag2_in = dram.tile([BLK, S], BF16, name="ag2_in")
            ag2_out = dram.tile([N_CORES * BLK, S], BF16, name="ag2_out",
                                addr_space="Shared")
            rs1_in = [dram.tile([HID, SC], BF16, name=f"rs1_in{sc}") for sc in range(NSC)]
            rs1_out = [dram.tile([HID_SH, SC], BF16, name=f"rs1_out{sc}")
                       for sc in range(NSC)]
            rs2_in = dram.tile([HID, S], BF16, name="rs2_in")
            rs2_out = dram.tile([HID_SH, S], BF16, name="rs2_out")

            ag1_v = ag1_out[:].rearrange("(c r) s -> c r s", r=BLK)
            ag2_v = ag2_out[:].rearrange("(c r) s -> c r s", r=BLK)
            wqkv_v = wqkv_c[:].rearrange("p (k c) -> p k c", k=NT_HID)
            wo_v = wo_c[:].rearrange("p (a c) -> p a c", a=HQ)
            wup_v = wup_c[:].rearrange("p (k c) -> p k c", k=NT_HID)

            def hg_src(ag_v, k, cs):
                # global hid tile k of the gathered (blocked) activation
                return ag_v[k // NT_HSH, (k % NT_HSH) * P:(k % NT_HSH + 1) * P, cs]

            # ============ rope tables (scoped scratch) ============
            with tc.tile_pool(name="tbl", bufs=1) as tbl:
                iot = tbl.tile([64, 1], I32)
                nc.gpsimd.iota(iot[:], pattern=[[1, 1]], base=0, channel_multiplier=1)
                iotf = tbl.tile([64, 1], F32)
                nc.vector.tensor_copy(iotf[:], iot[:])
                invf = tbl.tile([64, 1], F32)
                nc.scalar.activation(invf[:], iotf[:], AF.Exp,
                                     scale=-math.log(THETA) / 64.0)
                invf2pi = tbl.tile([64, 1], F32)
                nc.scalar.activation(invf2pi[:], invf[:], AF.Copy,
                                     scale=1.0 / TWO_PI)
                posi = tbl.tile([1, S], I32)
                nc.sync.dma_start(posi[:], pos_in[:])
                posf = tbl.tile([1, S], F32)
                nc.vector.tensor_copy(posf[:], posi[:])
                posb = tbl.tile([64, S], F32)
                nc.gpsimd.partition_broadcast(posb[:], posf[:])

                def range_reduce_sin(dst_bf, t_ap, negate=False):
                    # dst = sin(2*pi*t) via two-stage round-and-subtract
                    n1 = tbl.tile([64, S], I32, tag="rri", bufs=2, name="n1")
                    nc.vector.tensor_copy(n1[:], t_ap)
                    n1f = tbl.tile([64, S], F32, tag="rrf", bufs=2, name="n1f")
                    nc.vector.tensor_copy(n1f[:], n1[:])
                    f1 = tbl.tile([64, S], F32, tag="rrg", bufs=2, name="f1")
                    nc.vector.tensor_tensor(f1[:], t_ap, n1f[:], ALU.subtract)
                    n2 = tbl.tile([64, S], I32, tag="rri", bufs=2, name="n2")
                    nc.vector.tensor_copy(n2[:], f1[:])
                    n2f = tbl.tile([64, S], F32, tag="rrf", bufs=2, name="n2f")
                    nc.vector.tensor_copy(n2f[:], n2[:])
                    f2 = tbl.tile([64, S], F32, tag="rrg", bufs=2, name="f2")
                    nc.vector.tensor_tensor(f2[:], f1[:], n2f[:], ALU.subtract)
                    nc.scalar.activation(dst_bf, f2[:], AF.Sin,
                                         scale=-TWO_PI if negate else TWO_PI)

                tfrac = tbl.tile([64, S], F32)
                nc.scalar.activation(tfrac[:], posb[:], AF.Copy, scale=invf2pi[:])
                sinb = tbl.tile([64, S], BF16)
                sinnb = tbl.tile([64, S], BF16)
                range_reduce_sin(sinb[:], tfrac[:])
                range_reduce_sin(sinnb[:], tfrac[:], negate=True)
                tfrac2 = tbl.tile([64, S], F32)
                nc.scalar.activation(tfrac2[:], tfrac[:], AF.Copy, bias=0.25)
                cosb = tbl.tile([64, S], BF16)
                range_reduce_sin(cosb[:], tfrac2[:])
                nc.sync.dma_start(cos2[:64, :], cosb[:])
                nc.sync.dma_start(cos2[64:, :], cosb[:])
                nc.sync.dma_start(sin_neg[:64, :], sinnb[:])
                nc.sync.dma_start(sin_neg[64:, :], sinb[:])

            # ================== attention era ==================
            with tc.tile_pool(name="apersist", bufs=1) as apersist, \
                 tc.tile_pool(name="awork", bufs=1) as awork, \
                 tc.tile_pool(name="wstr", bufs=1) as wstr:

                _cnt = [0]

                def t2k(tag="t2k", bufs=9):
                    _cnt[0] += 1
                    return awork.tile([P, SC], F32, tag=tag, bufs=bufs,
                                      name=f"t_{_cnt[0]}")

                def t1k(tag="t1k", bufs=8):
                    _cnt[0] += 1
                    return awork.tile([P, SC], BF16, tag=tag, bufs=bufs,
                                      name=f"t_{_cnt[0]}")

                # ---- phase 1: x = h + r (chunked), ssq row, ag1_in (bf16)
                ssq1 = awork.tile([1, S], F32, name="ssq1")
                for sc in range(NSC):
                    cs = slice(sc * SC, (sc + 1) * SC)
                    ps = rowps.tile([1, SC], F32, tag="row", name=f"ssq1p{sc}")
                    for i in range(NT_HSH):
                        a = t2k()
                        b = t2k()
                        nc.sync.dma_start(a[:], hT[i * P:(i + 1) * P, cs])
                        nc.sync.dma_start(b[:], rT[i * P:(i + 1) * P, cs])
                        xt = t2k()
                        nc.vector.tensor_tensor(xt[:], a[:], b[:], ALU.add)
                        nc.sync.dma_start(xT_d[i * P:(i + 1) * P, cs], xt[:])
                        xb = t1k()
                        nc.vector.tensor_copy(xb[:], xt[:])
                        nc.sync.dma_start(ag1_in[i * P:(i + 1) * P, cs], xb[:])
                        sq = t1k(tag="sq", bufs=4)
                        nc.scalar.activation(sq[:], xt[:], AF.Square)
                        nc.tensor.matmul(ps[:], ones_bf[:], sq[:],
                                         start=(i == 0), stop=(i == NT_HSH - 1))
                    nc.vector.tensor_copy(ssq1[:, cs], ps[:])
                ssq1b = awork.tile([1, S], BF16, name="ssq1b")
                nc.vector.tensor_copy(ssq1b[:], ssq1[:])
                nc.sync.dma_start(ag1_in[HID_SH:HID_SH + 1, :], ssq1b[:])
                nc.gpsimd.collective_compute("AllGather", ALU.bypass, replica_groups=RG,
                                             ins=[ag1_in[:].opt()],
                                             outs=[ag1_out[:].opt()])

                # ---- weight caches (emitted after AG1 so phase-1 DMAs win)
                def build_cache(src, n_row_tiles, n_cols, dst, ln_col, eng):
                    CB = min(n_cols, 1024)
                    for k in range(n_row_tiles):
                        for c0 in range(0, n_cols, CB):
                            wf = wstr.tile([P, CB], F32, tag="cbf", bufs=2,
                                           name=f"cb_{dst.tensor.name}_{k}_{c0}")
                            nc.scalar.dma_start(wf[:, :min(CB, n_cols - c0)],
                                                src[k * P:(k + 1) * P,
                                                    c0:c0 + min(CB, n_cols - c0)])
                            wb = wstr.tile([P, CB], BF16, tag="cbb", bufs=2,
                                           name=f"cc_{dst.tensor.name}_{k}_{c0}")
                            w = min(CB, n_cols - c0)
                            if ln_col is not None:
                                nc.scalar.activation(wb[:, :w], wf[:, :w], AF.Copy,
                                                     scale=ln_col[:, k:k + 1])
                            else:
                                eng.tensor_copy(wb[:, :w], wf[:, :w])
                            nc.scalar.dma_start(
                                dst[:, k * n_cols + c0:k * n_cols + c0 + w],
                                wb[:, :w])

                build_cache(wqkv, NT_HID, QKV_COLS, wqkv_c, ln1_sb, None)
                build_cache(wo, HQ, HID, wo_c, None, nc.gpsimd)

                # per-chunk norm1 scale rows (from gathered ssq partials)
                s1b = apersist.tile([P, S], F32, name="s1b")  # rsqrt scale bcast
                for sc in range(NSC):
                    cs = slice(sc * SC, (sc + 1) * SC)
                    srows_b = awork.tile([8, SC], BF16, tag="srb", bufs=1,
                                         name=f"sr1b{sc}")
                    nc.sync.dma_start(srows_b[:], ag1_v[:, HID_SH, cs])
                    srows = awork.tile([8, SC], F32, tag="srf", bufs=1,
                                       name=f"sr1f{sc}")
                    nc.vector.tensor_copy(srows[:], srows_b[:])
                    ssum = awork.tile([8, SC], F32, tag="ssum", bufs=1,
                                      name=f"ss1{sc}")
                    nc.gpsimd.partition_all_reduce(ssum[:], srows[:], channels=8,
                                                   reduce_op=ReduceOp.add)
                    var = awork.tile([1, SC], F32, tag="var", bufs=2, name=f"v1{sc}")
                    nc.scalar.activation(var[:], ssum[:1, :], AF.Copy,
                                         scale=1.0 / HID, bias=EPS)
                    nc.vector.reciprocal(var[:], var[:])
                    nc.scalar.activation(var[:], var[:], AF.Sqrt)
                    nc.gpsimd.partition_broadcast(s1b[:, cs], var[:])

                qkvT = [apersist.tile([P, S], BF16, name=f"qkvT{j}") for j in range(NJ)]
                attnT = [apersist.tile([P, S], BF16, name=f"attnT{a}") for a in range(HQ)]

                for sc in range(NSC):
                    cs = slice(sc * SC, (sc + 1) * SC)
                    # ---- qkv chunk (scale1 folded into eviction)
                    hg = []
                    for k in range(NT_HID):
                        g = awork.tile([P, SC], BF16, tag=f"hg{k}", bufs=1,
                                       name=f"hg{k}_{sc}")
                        nc.sync.dma_start(g[:], hg_src(ag1_v, k, cs))
                        hg.append(g)
                    for j in range(NJ):
                        KH = NT_HID // 2
                        wj0 = wstr.tile([P, KH, P], BF16, tag="wqkvs", bufs=3,
                                        name=f"wj{j}_{sc}a")
                        nc.scalar.dma_start(wj0[:], wqkv_v[:, :KH, j * P:(j + 1) * P])
                        wj1 = wstr.tile([P, KH, P], BF16, tag="wqkvs", bufs=3,
                                        name=f"wj{j}_{sc}b")
                        nc.scalar.dma_start(wj1[:], wqkv_v[:, KH:, j * P:(j + 1) * P])
                        ps = acc.tile([P, SC], F32, tag="acc", name=f"qk{j}_{sc}")
                        for k in range(NT_HID):
                            wk = wj0[:, k, :] if k < KH else wj1[:, k - KH, :]
                            nc.tensor.matmul(ps[:], wk, hg[k][:],
                                             start=(k == 0), stop=(k == NT_HID - 1))
                        nc.vector.tensor_tensor(qkvT[j][:, cs], ps[:], s1b[:, cs],
                                                ALU.mult)

                    # ---- rope on q tiles and k tile (bf16, chunk cols)
                    for j in range(HQ + 1):
                        t = qkvT[j]
                        swp = t1k()
                        nc.sync.dma_start(swp[:64, :], t[64:, cs])
                        nc.sync.dma_start(swp[64:, :], t[:64, cs])
                        m1 = t1k()
                        nc.vector.tensor_tensor(m1[:], t[:, cs], cos2[:, cs], ALU.mult)
                        m2 = t1k()
                        nc.vector.tensor_tensor(m2[:], swp[:], sin_neg[:, cs], ALU.mult)
                        nc.vector.tensor_tensor(t[:, cs], m1[:], m2[:], ALU.add)

                    # ---- v transpose in place (block-transposed v)
                    for t in range(sc * (SC // P), (sc + 1) * (SC // P)):
                        pst = tpps.tile([P, P], BF16, tag="tp", name=f"tp{t}")
                        nc.tensor.transpose(pst[:], qkvT[NJ - 1][:, t * P:(t + 1) * P],
                                            ident[:])
                        nc.vector.tensor_copy(qkvT[NJ - 1][:, t * P:(t + 1) * P], pst[:])

                    # ---- attention (4 heads x this chunk)
                    nsk = (sc + 1) * (SC // P)
                    for h in range(HQ):
                        pv = acc.tile([P, SC], F32, tag="acc", name=f"pv{h}_{sc}")
                        rs = rowps.tile([1, SC], F32, tag="row", name=f"rs{h}_{sc}")
                        for skt in range(nsk):
                            sps = acc.tile([P, SC], F32, tag="acc",
                                           name=f"s{h}_{sc}_{skt}")
                            nc.tensor.matmul(sps[:],
                                             qkvT[HQ][:, skt * P:(skt + 1) * P],
                                             qkvT[h][:, cs], start=True, stop=True)
                            ex = t1k(tag="ex", bufs=5)
                            nc.scalar.activation(ex[:], sps[:], AF.Exp,
                                                 scale=inv_sqrt_d)
                            if skt >= 4 * sc:
                                nc.gpsimd.affine_select(
                                    ex[:], ex[:], pattern=[[1, SC]],
                                    base=sc * SC - skt * P, channel_multiplier=-1,
                                    compare_op=ALU.is_ge, fill=0.0)
                            nc.tensor.matmul(rs[:], ones_bf[:], ex[:],
                                             start=(skt == 0), stop=(skt == nsk - 1))
                            nc.tensor.matmul(pv[:],
                                             qkvT[NJ - 1][:, skt * P:(skt + 1) * P],
                                             ex[:], start=(skt == 0),
                                             stop=(skt == nsk - 1))
                        rcp = awork.tile([1, SC], F32, tag="rcp", bufs=2,
                                         name=f"rcp{h}_{sc}")
                        nc.vector.reciprocal(rcp[:], rs[:])
                        rcpb = t2k(tag="rcpb", bufs=2)
                        nc.gpsimd.partition_broadcast(rcpb[:], rcp[:])
                        nc.vector.tensor_tensor(attnT[h][:, cs], pv[:], rcpb[:],
                                                ALU.mult)

                    # ---- o_proj chunk -> bf16 ReduceScatter
                    for m in range(NT_HID):
                        wm = wstr.tile([P, HQ, P], BF16, tag="wos", bufs=3,
                                       name=f"wm{m}_{sc}")
                        nc.scalar.dma_start(wm[:], wo_v[:, :, m * P:(m + 1) * P])
                        ps = acc.tile([P, SC], F32, tag="acc", name=f"o{m}_{sc}")
                        for a in range(HQ):
                            nc.tensor.matmul(ps[:], wm[:, a, :], attnT[a][:, cs],
                                             start=(a == 0), stop=(a == HQ - 1))
                        ev = t1k(tag="oev", bufs=3)
                        nc.vector.tensor_copy(ev[:], ps[:])
                        nc.sync.dma_start(rs1_in[sc][m * P:(m + 1) * P, :], ev[:])
                    nc.gpsimd.collective_compute(
                        "ReduceScatter", ALU.add, replica_groups=RG,
                        ins=[rs1_in[sc][:].opt()], outs=[rs1_out[sc][:].opt()])

                    # ---- residual2 chunk -> raw bf16 + ssq row into ag2_in
                    ps2 = rowps.tile([1, SC], F32, tag="row", name=f"ssq2_{sc}")
                    for i in range(NT_HSH):
                        o = t1k(tag="r2ld", bufs=3)
                        nc.sync.dma_start(o[:], rs1_out[sc][i * P:(i + 1) * P, :])
                        xr = t2k()
                        nc.sync.dma_start(xr[:], xT_d[i * P:(i + 1) * P, cs])
                        r2t = t2k(tag="r2", bufs=5)
                        nc.vector.tensor_tensor(r2t[:], o[:], xr[:], ALU.add)
                        nc.sync.dma_start(out_res2[i * P:(i + 1) * P, cs], r2t[:])
                        r2b = t1k()
                        nc.vector.tensor_copy(r2b[:], r2t[:])
                        nc.sync.dma_start(ag2_in[i * P:(i + 1) * P, cs], r2b[:])
                        sq = t1k(tag="sq", bufs=4)
                        nc.scalar.activation(sq[:], r2t[:], AF.Square)
                        nc.tensor.matmul(ps2[:], ones_bf[:], sq[:],
                                         start=(i == 0), stop=(i == NT_HSH - 1))
                    ssq2 = awork.tile([1, SC], BF16, tag="ssq2", bufs=2,
                                      name=f"sq2_{sc}")
                    nc.vector.tensor_copy(ssq2[:], ps2[:])
                    nc.sync.dma_start(ag2_in[HID_SH:HID_SH + 1, cs], ssq2[:])

                build_cache(wup, NT_HID, INT_SH, wup_c, ln2_sb, None)
                nc.gpsimd.collective_compute("AllGather", ALU.bypass, replica_groups=RG,
                                             ins=[ag2_in[:].opt()],
                                             outs=[ag2_out[:].opt()])

            # ================== MLP era ==================
            with tc.tile_pool(name="mpersist", bufs=1) as mpersist, \
                 tc.tile_pool(name="mwork", bufs=1) as mwork, \
                 tc.tile_pool(name="mstr", bufs=1) as mstr:
                # per-chunk 1/var rows (scale2^2 fold for down eviction)
                s2b = mpersist.tile([P, S], F32, name="s2b")
                for sc in range(NSC):
                    cs = slice(sc * SC, (sc + 1) * SC)
                    srows_b = mwork.tile([8, SC], BF16, tag="srb", bufs=1,
                                         name=f"sr2b{sc}")
                    nc.sync.dma_start(srows_b[:], ag2_v[:, HID_SH, cs])
                    srows = mwork.tile([8, SC], F32, tag="srf", bufs=1,
                                       name=f"sr2f{sc}")
                    nc.vector.tensor_copy(srows[:], srows_b[:])
                    ssum = mwork.tile([8, SC], F32, tag="ssum", bufs=1,
                                      name=f"ss2{sc}")
                    nc.gpsimd.partition_all_reduce(ssum[:], srows[:], channels=8,
                                                   reduce_op=ReduceOp.add)
                    var = mwork.tile([1, SC], F32, tag="var", bufs=2, name=f"v2{sc}")
                    nc.scalar.activation(var[:], ssum[:1, :], AF.Copy,
                                         scale=1.0 / HID, bias=EPS)
                    nc.vector.reciprocal(var[:], var[:])  # = scale2^2
                    nc.gpsimd.partition_broadcast(s2b[:, cs], var[:])

                uT = [mpersist.tile([P, S], BF16, name=f"uT{it}")
                      for it in range(NT_INT)]
                for half in range(2):
                    hs = [half * 2, half * 2 + 1]
                    h2g = []
                    for k in range(NT_HID):
                        g = mwork.tile([P, 2 * SC], BF16, tag=f"h2g{k}", bufs=1,
                                       name=f"h2g{k}_{half}")
                        for ci, sc_ in enumerate(hs):
                            nc.sync.dma_start(
                                g[:, ci * SC:(ci + 1) * SC],
                                hg_src(ag2_v, k, slice(sc_ * SC, (sc_ + 1) * SC)))
                        h2g.append(g)
                    for it in range(NT_INT):
                        KH = NT_HID // 2
                        wt0 = mstr.tile([P, KH, P], BF16, tag="wups", bufs=3,
                                        name=f"wt{it}_{half}a")
                        nc.scalar.dma_start(wt0[:], wup_v[:, :KH, it * P:(it + 1) * P])
                        wt1 = mstr.tile([P, KH, P], BF16, tag="wups", bufs=3,
                                        name=f"wt{it}_{half}b")
                        nc.scalar.dma_start(wt1[:], wup_v[:, KH:, it * P:(it + 1) * P])
                        for ci, sc_ in enumerate(hs):
                            ps = acc.tile([P, SC], F32, tag="acc",
                                          name=f"up{it}_{sc_}")
                            for k in range(NT_HID):
                                wk = wt0[:, k, :] if k < KH else wt1[:, k - KH, :]
                                nc.tensor.matmul(ps[:], wk,
                                                 h2g[k][:, ci * SC:(ci + 1) * SC],
                                                 start=(k == 0),
                                                 stop=(k == NT_HID - 1))
                            rl = mwork.tile([P, SC], F32, tag="relu", bufs=2,
                                            name=f"rl{it}_{sc_}")
                            nc.scalar.activation(rl[:], ps[:], AF.Relu)
                            nc.vector.tensor_tensor(
                                uT[it][:, sc_ * SC:(sc_ + 1) * SC], rl[:], rl[:],
                                ALU.mult)

                for m in range(NT_HID):
                    wdn_t = []
                    for it in range(NT_INT):
                        wf = mstr.tile([P, P], F32, tag="wdnf", bufs=4,
                                       name=f"wf{m}_{it}")
                        nc.scalar.dma_start(
                            wf[:], wdn[it * P:(it + 1) * P, m * P:(m + 1) * P])
                        wb = mstr.tile([P, P], BF16, tag="wdnb", bufs=NT_INT + 2,
                                       name=f"wb{m}_{it}")
                        nc.gpsimd.tensor_copy(wb[:], wf[:])
                        wdn_t.append(wb)
                    for sc in range(NSC):
                        cs = slice(sc * SC, (sc + 1) * SC)
                        ps = acc.tile([P, SC], F32, tag="acc", name=f"dn{m}_{sc}")
                        for it in range(NT_INT):
                            nc.tensor.matmul(ps[:], wdn_t[it][:], uT[it][:, cs],
                                             start=(it == 0), stop=(it == NT_INT - 1))
                        ev = mwork.tile([P, SC], BF16, tag="dnev", bufs=3,
                                        name=f"dev{m}_{sc}")
                        nc.vector.tensor_tensor(ev[:], ps[:], s2b[:, cs], ALU.mult)
                        nc.sync.dma_start(rs2_in[m * P:(m + 1) * P, cs], ev[:])

                nc.gpsimd.collective_compute(
                    "ReduceScatter", ALU.add, replica_groups=RG,
                    ins=[rs2_in[:].opt()], outs=[rs2_out[:].opt()])
                for i in range(NT_HSH):
                    for sc in range(NSC):
                        cs = slice(sc * SC, (sc + 1) * SC)
                        mo = mwork.tile([P, SC], BF16, tag="mold", bufs=2,
                                        name=f"mo{i}_{sc}")
                        nc.sync.dma_start(mo[:], rs2_out[i * P:(i + 1) * P, cs])
                        mof = mwork.tile([P, SC], F32, tag="mof", bufs=2,
                                         name=f"mf{i}_{sc}")
                        nc.vector.tensor_copy(mof[:], mo[:])
                        nc.sync.dma_start(out_mlp[i * P:(i + 1) * P, cs], mof[:])

    nc.compile()
    return nc


def shard_inputs(positions, hidden_states, residual, qkv_w, o_w, up_w, down_w,
                 ln1_w, ln2_w):
    hTf = np.ascontiguousarray(np.asarray(hidden_states).reshape(S, HID).T)
    rTf = np.ascontiguousarray(np.asarray(residual).reshape(S, HID).T)
    pos = np.ascontiguousarray(np.asarray(positions).reshape(1, S))
    ln1_t = np.ascontiguousarray(np.asarray(ln1_w).reshape(NT_HID, P).T)  # [128,32]
    ln2_t = np.ascontiguousarray(np.asarray(ln2_w).reshape(NT_HID, P).T)
    q_size = N_HEADS * DHEAD
    kv = N_KV * DHEAD
    in_maps = []
    for c in range(N_CORES):
        wqkv_c = np.concatenate([
            qkv_w[:, c * HQ * DHEAD:(c + 1) * HQ * DHEAD],
            qkv_w[:, q_size + c * DHEAD:q_size + (c + 1) * DHEAD],
            qkv_w[:, q_size + kv + c * DHEAD:q_size + kv + (c + 1) * DHEAD],
        ], axis=1)
        in_maps.append({
            "hT": np.ascontiguousarray(hTf[c * HID_SH:(c + 1) * HID_SH]),
            "rT": np.ascontiguousarray(rTf[c * HID_SH:(c + 1) * HID_SH]),
            "positions": pos,
            "wqkv": np.ascontiguousarray(wqkv_c),
            "wo": np.ascontiguousarray(o_w[c * HQ * DHEAD:(c + 1) * HQ * DHEAD, :]),
            "wup": np.ascontiguousarray(up_w[:, c * INT_SH:(c + 1) * INT_SH]),
            "wdn": np.ascontiguousarray(down_w[c * INT_SH:(c + 1) * INT_SH, :]),
            "ln1": ln1_t,
            "ln2": ln2_t,
        })
    return in_maps


_CACHE = {}


def kernel(**inputs):
    from concourse.bass_utils import run_bass_kernel_spmd
    if "nc" not in _CACHE:
        _CACHE["nc"] = build_graph()
    nc = _CACHE["nc"]
    in_maps = shard_inputs(**{k: np.asarray(v) for k, v in inputs.items()})
    res = run_bass_kernel_spmd(nc, in_maps, core_ids=list(range(N_CORES)),
                               trace=False)
    mlpT = np.concatenate([res.results[c]["mlpT"] for c in range(N_CORES)], axis=0)
    res2T = np.concatenate([res.results[c]["res2T"] for c in range(N_CORES)], axis=0)
    mlp_out = np.ascontiguousarray(mlpT.T).reshape(1, S, HID)
    residual2 = np.ascontiguousarray(res2T.T).reshape(1, S, HID)
    return mlp_out, residual2
